# revision 69
# baseline (speedup 1.0000x reference)
"""Multi-head attention (S=2048, B=2, D=1024, H=16) on 8 Trainium2 NeuronCores.

Sharding: batch x head-group. Core c handles batch c//4 and heads
[4*(c%4), 4*(c%4)+4). Each core computes its 4 heads' Q/K/V projections,
attention, and a partial output projection (row-parallel Wo); the host sums
the 4 partials per batch and adds the bias terms (bo and the exact wo@bv
correction; softmax rows sum to 1 so bv folds out of the attention).

Device-side structure (per core):
  - x inputs and all weights travel as bf16; all x DMAs are issued eagerly
    at build time in demand order into persistent SBUF.
  - QT/KT (dk-major, 2 tiles of (128, S)): one head pair per tile, f32r.
  - scores computed transposed, ST = (j, i), via fp32r matmuls; the two
    heads of a pair run in disjoint PE row groups (K=dk=64 each); exp on
    ScalarE straight out of PSUM with the 1/sqrt(dk) scale folded in.
  - the attention sweep runs over 32 four-tick blocks, one (i-block,
    head-pair) x (512-key segment) each, in a STAIRCASE order: early pairs
    finish their full softmax row early (first finish ~tick 50) while late
    pairs start late. This spreads the K/V projection demand over the first
    half of the run AND spreads the output-projection work over the second
    half, so neither the head (projection crunch) nor the tail (phase-C
    crunch) over-subscribes an engine.
  - between segments a pair's partial U accumulator (one PSUM bank,
    [128, 512] = 2 heads x 4 i-tiles x 64) is spilled to SBUF as bf16 and
    later reloaded by a single identity matmul (start=True reopens the
    accumulation group). The softmax denominators Z accumulate separately
    via one-row matmuls (et^T @ ones) into a persistent PSUM bank that is
    never spilled.
  - PSUM banks: 2x2 score double-buffer (exclusively scores -> the
    PE->ACT->PE pipeline double-buffers cleanly), 2x1 U blocks, 1 Z bank,
    1 scratch bank (projection PSUM / output-projection / transposes).
  - O = U * (1/Z) per query row, fused into the PSUM->SBUF copy on the
    otherwise-idle Pool engine; PE-transposed in bf16; output projection
    in bf16; y staged bf16 and DMA'd per 512-column half.
  - projections and phase-C steps are emitted through a worklist drained
    inside the attention tick loop (one heavy item per two ticks, every
    tick near the end) so DMA/PE/ACT/DVE/Pool overlap end to end.
"""

import sys

sys.path.insert(0, "/opt/trn_rl_repo")

from collections import deque

import ml_dtypes
import numpy as np

import concourse.bass as bass
import concourse.tile as tile
from concourse import bacc, mybir
from concourse.bass_utils import run_bass_kernel_spmd
from concourse.masks import make_identity

S = 2048
B = 2
D = 1024
H = 16
DK = 64
G = 4            # heads per core
DC = G * DK      # 256 per-core head dims
SCALE = 1.0 / np.sqrt(DK)  # 0.125
P = 128
NJ = S // P      # 16 key chunks
NIT = S // P     # 16 query tiles
NIB = 4          # i blocks of 512
IB = S // NIB    # 512
ND = D // P      # 8 contraction chunks for projections
NSEG = 4         # j segments (one 512-key block each)
NP = 8           # (i-block, head-pair) pairs

F32 = mybir.dt.float32
F32R = mybir.dt.float32r
BF16 = mybir.dt.bfloat16
FP16 = mybir.dt.float16
EXP = mybir.ActivationFunctionType.Exp
CPY = mybir.ActivationFunctionType.Copy
ADD = mybir.AluOpType.add
MULT = mybir.AluOpType.mult

# staircase block order: (pair, seg); pair p = 2*ib + hp
BLOCKS = [
    (0, 0), (1, 0), (2, 0), (3, 0),
    (0, 1), (4, 0), (1, 1), (5, 0),
    (0, 2), (2, 1), (6, 0), (3, 1),
    (0, 3), (1, 2), (7, 0), (4, 1),
    (1, 3), (2, 2), (5, 1), (3, 2),
    (2, 3), (6, 1), (4, 2), (3, 3),
    (5, 2), (7, 1), (4, 3), (6, 2),
    (5, 3), (6, 3), (7, 2), (7, 3),
]


def _check_blocks():
    seen = {}
    for b, (p, s) in enumerate(BLOCKS):
        assert seen.get(p, -1) == s - 1, f"pair {p} segment order broken at {b}"
        seen[p] = s
    assert all(s == NSEG - 1 for s in seen.values())


_check_blocks()

_NC_CACHE = None


def _build():
    nc = bacc.Bacc("TRN2", target_bir_lowering=False, debug=False)

    xq_t = nc.dram_tensor("xq_t", [D, S], BF16, kind="ExternalInput")
    xk_t = nc.dram_tensor("xk_t", [D, S], BF16, kind="ExternalInput")
    xv_t = nc.dram_tensor("xv_t", [D, S], BF16, kind="ExternalInput")
    wq_t = nc.dram_tensor("wq_t", [D, DC], BF16, kind="ExternalInput")
    wk_t = nc.dram_tensor("wk_t", [D, DC], BF16, kind="ExternalInput")
    wv_t = nc.dram_tensor("wv_t", [D, DC], BF16, kind="ExternalInput")
    wo_t = nc.dram_tensor("wo_t", [DC, D], BF16, kind="ExternalInput")
    bqk_s = nc.dram_tensor("bqk_s", [P, 4], F32, kind="ExternalInput")
    y = nc.dram_tensor("y", [S, D], BF16, kind="ExternalOutput")

    with tile.TileContext(nc) as tc:
        with (
            tc.tile_pool(name="persist", bufs=1) as persist,
            tc.tile_pool(name="stp", bufs=2, space="PSUM") as stp,  # scores only
            tc.tile_pool(name="up", bufs=2, space="PSUM") as up,    # U pair blocks
            tc.tile_pool(name="zp", bufs=1, space="PSUM") as zp,    # Z accumulators
            tc.tile_pool(name="scp", bufs=1, space="PSUM") as scp,  # proj/oproj/tp
            tc.tile_pool(name="et", bufs=8) as etp,
            tc.tile_pool(name="rz", bufs=4) as rzp,
            tc.tile_pool(name="ysb", bufs=4) as ysb,
        ):
            # ---- persistent SBUF (DMAs issued in first-demand order) ----
            wq_sb = persist.tile([P, ND, DC], BF16)
            wk_sb = persist.tile([P, ND, DC], BF16)
            wv_sb = persist.tile([P, ND, DC], BF16)
            bqk_sb = persist.tile([P, 4], F32)
            bq_sb = bqk_sb[:, 0:2]
            bk_sb = bqk_sb[:, 2:4]
            woc_sb = persist.tile([P, 2, D], BF16)

            x_all = {
                "k": persist.tile([P, ND, S], BF16, tag="xk", name="xk_all"),
                "q": persist.tile([P, ND, S], BF16, tag="xq", name="xq_all"),
                "v": persist.tile([P, ND, S], BF16, tag="xv", name="xv_all"),
            }
            xsrc = {"k": xk_t, "q": xq_t, "v": xv_t}

            qt_sb = [persist.tile([P, S], F32R, tag=f"qt{t}", name=f"qt{t}") for t in range(2)]
            kt_sb = [persist.tile([P, S], F32R, tag=f"kt{t}", name=f"kt{t}") for t in range(2)]
            vt_sb = persist.tile([P, NJ, DC], FP16)
            ones_sb = persist.tile([P, 1], FP16)
            usp = persist.tile([P, NP, 2 * DC], F32R)  # spilled partial U (f32r)
            o_sb = persist.tile([P, NIT, DC], BF16)
            ot_sb = [persist.tile([P, S], BF16, tag=f"ot{t}", name=f"ot{t}") for t in range(2)]
            ident_f = persist.tile([P, P], F32)
            ident = persist.tile([P, P], BF16)
            ident_r = persist.tile([P, P], F32R)

            def dma_slab(key, cb):
                nc.sync.dma_start(
                    out=x_all[key][:, :, cb * IB : (cb + 1) * IB],
                    in_=xsrc[key].ap()[:, cb * IB : (cb + 1) * IB].rearrange(
                        "(c p) m -> p c m", p=P
                    ),
                )

            # eager DMA stream in first-demand order
            def dma_w(dst, srct, mt):
                nc.sync.dma_start(
                    out=dst[:, :, mt * P : (mt + 1) * P],
                    in_=srct.ap()[:, mt * P : (mt + 1) * P].rearrange(
                        "(c p) m -> p c m", p=P
                    ),
                )

            dma_w(wk_sb, wk_t, 0)
            dma_slab("k", 0)
            nc.sync.dma_start(out=bqk_sb, in_=bqk_s.ap())
            dma_w(wq_sb, wq_t, 0)
            dma_slab("q", 0)
            dma_w(wk_sb, wk_t, 1)
            dma_w(wq_sb, wq_t, 1)
            nc.sync.dma_start(out=wv_sb, in_=wv_t.ap().rearrange("(c p) m -> p c m", p=P))
            dma_slab("v", 0)
            dma_slab("q", 1)
            dma_slab("k", 1)
            dma_slab("v", 1)
            dma_slab("q", 2)
            dma_slab("k", 2)
            dma_slab("v", 2)
            dma_slab("q", 3)
            dma_slab("k", 3)
            dma_slab("v", 3)
            nc.sync.dma_start(out=woc_sb, in_=wo_t.ap().rearrange("(t p) n -> p t n", p=P))

            nc.vector.memset(ones_sb, 1.0)
            make_identity(nc, ident_f)
            nc.vector.tensor_copy(ident, ident_f)
            nc.vector.tensor_copy(ident_r, ident_f)

            # persistent Z bank: zeroed once; all Z matmuls accumulate (start=False)
            z_ps = zp.tile([P, NP, 8], F32, tag="z", name="z_ps")
            nc.vector.memset(z_ps, 0.0)

            emitted = set()
            HW_ = IB // 2

            def proj_qk(key, cb, mt, half, w_sb, b_sb, out_tiles):
                ps = scp.tile([P, HW_], F32, tag="sc", name="ps")
                for dc in range(ND):
                    nc.tensor.matmul(
                        ps,
                        w_sb[:, dc, mt * P : (mt + 1) * P],
                        x_all[key][:, dc, cb * IB + half * HW_ : cb * IB + (half + 1) * HW_],
                        start=(dc == 0),
                        stop=(dc == ND - 1),
                    )
                nc.vector.tensor_scalar(
                    out_tiles[mt][:, cb * IB + half * HW_ : cb * IB + (half + 1) * HW_],
                    ps,
                    b_sb[:, mt : mt + 1],
                    None,
                    op0=ADD,
                )
                emitted.add((key, cb, mt, half))

            def proj_v(cb, jq):
                jt = cb * 4 + jq
                ps = scp.tile([P, DC], F32, tag="sc", name="ps")
                for dc in range(ND):
                    nc.tensor.matmul(
                        ps,
                        x_all["v"][:, dc, jt * P : (jt + 1) * P],
                        wv_sb[:, dc, :],
                        start=(dc == 0),
                        stop=(dc == ND - 1),
                    )
                nc.vector.tensor_copy(vt_sb[:, jt], ps)
                emitted.add(("v", cb, jq))

            def oproj(it, nh, late=False):
                pool, tg = (stp, "st") if late else (scp, "sc")
                yp = pool.tile([P, IB], F32, tag=tg, name="yp")
                for mt in range(2):
                    nc.tensor.matmul(
                        yp,
                        ot_sb[mt][:, it * P : (it + 1) * P],
                        woc_sb[:, mt, nh * IB : (nh + 1) * IB],
                        start=(mt == 0),
                        stop=(mt == 1),
                    )
                ys = ysb.tile([P, IB], BF16, tag="ysb", name="ysb")
                if late and nh == 0:
                    nc.scalar.activation(ys, yp, CPY)
                else:
                    nc.vector.tensor_copy(ys, yp)
                nc.sync.dma_start(
                    out=y.ap()[it * P : (it + 1) * P, nh * IB : (nh + 1) * IB],
                    in_=ys,
                )

            def transp2(ib, mt, pair, late=False):
                pool, tg = (stp, "st") if late else (scp, "sc")
                for it in range(ib * 4 + 2 * pair, ib * 4 + 2 * pair + 2):
                    tp = pool.tile([P, P], BF16, tag=tg, name="tp")
                    nc.tensor.transpose(
                        tp, o_sb[:, it, mt * P : (mt + 1) * P], ident
                    )
                    nc.vector.tensor_copy(
                        ot_sb[mt][:, it * P : (it + 1) * P], tp
                    )

            work = deque()

            def drain(tick, force=False):
                k = 2 if tick < 80 else 3
                if work and (tick % k == 0 or force):
                    work.popleft()()

            def drain_until(key):
                while key not in emitted:
                    assert work, f"work exhausted before {key}"
                    work.popleft()()

            # fill: pair 0 = (ib0, hp0) first
            proj_qk("k", 0, 0, 0, wk_sb, bk_sb, kt_sb)
            proj_qk("k", 0, 0, 1, wk_sb, bk_sb, kt_sb)
            proj_qk("q", 0, 0, 0, wq_sb, bq_sb, qt_sb)
            proj_qk("q", 0, 0, 1, wq_sb, bq_sb, qt_sb)

            # worklist ordered by staircase demand
            def wq_item(ib, mt, half):
                work.append(lambda: proj_qk("q", ib, mt, half, wq_sb, bq_sb, qt_sb))

            def wk_item(cb, mt, half):
                work.append(lambda: proj_qk("k", cb, mt, half, wk_sb, bk_sb, kt_sb))

            def wv_item(cb):
                for jq in range(4):
                    work.append(lambda jq=jq: proj_v(cb, jq))

            wk_item(0, 1, 0); wk_item(0, 1, 1)
            wq_item(0, 1, 0); wq_item(0, 1, 1)
            wv_item(0)
            wq_item(1, 0, 0); wq_item(1, 0, 1)
            wq_item(1, 1, 0); wq_item(1, 1, 1)
            wk_item(1, 0, 0); wk_item(1, 0, 1)
            wq_item(2, 0, 0); wq_item(2, 0, 1)
            wv_item(1)
            wk_item(1, 1, 0); wk_item(1, 1, 1)
            wq_item(2, 1, 0); wq_item(2, 1, 1)
            wk_item(2, 0, 0); wk_item(2, 0, 1)
            wv_item(2)
            wq_item(3, 0, 0); wq_item(3, 0, 1)
            wk_item(2, 1, 0); wk_item(2, 1, 1)
            wq_item(3, 1, 0); wq_item(3, 1, 1)
            wk_item(3, 0, 0); wk_item(3, 0, 1)
            wv_item(3)
            wk_item(3, 1, 0); wk_item(3, 1, 1)

            # ---- attention ticks ----
            seq = []
            for b, (p, s) in enumerate(BLOCKS):
                ib, hp = divmod(p, 2)
                for Jq in range(4):
                    seq.append((p, ib, hp, s * 4 + Jq))

            first_of_seg = {}
            for idx, (p, ib, hp, J) in enumerate(seq):
                first_of_seg.setdefault((J // 4, hp), idx)   # kt demand
            first_pv_of_seg = {}
            for idx, (p, ib, hp, J) in enumerate(seq):
                first_pv_of_seg.setdefault(J, idx)           # vt demand

            u_tiles = {}   # p -> u pair tile
            spilled = set()  # pairs with a live usp partial
            et_tiles = {}  # tick index -> et tile

            def emit_st_exp(idx):
                p, ib, hp, J = seq[idx]
                if J // 4 == 0 and J % 4 == 0:
                    drain_until(("q", ib, hp, 0))
                    drain_until(("q", ib, hp, 1))
                if first_of_seg[(J // 4, hp)] == idx:
                    drain_until(("k", J // 4, hp, 0))
                    drain_until(("k", J // 4, hp, 1))
                st = stp.tile([P, 2 * IB], F32, tag="st", name="st")
                for hx in range(2):
                    nc.tensor.matmul(
                        st[:, hx * IB : (hx + 1) * IB],
                        kt_sb[hp][hx * DK : (hx + 1) * DK, J * P : (J + 1) * P],
                        qt_sb[hp][hx * DK : (hx + 1) * DK, ib * IB : (ib + 1) * IB],
                        start=True,
                        stop=True,
                        tile_position=(hx * DK, 0),
                    )
                et = etp.tile([P, 2 * IB], FP16, tag="et", name="et")
                nc.scalar.activation(et, st, EXP, scale=float(SCALE))
                et_tiles[idx] = et

            def emit_pv(idx):
                p, ib, hp, J = seq[idx]
                seg = J // 4
                first = seg < NSEG - 1 and J % 4 == 0
                seg_end = J % 4 == 3
                last = seg == NSEG - 1 and seg_end
                if J % 4 == 0 and p not in u_tiles:
                    u = up.tile([P, 2 * DC], F32, tag="u", name="u")
                    u_tiles[p] = u
                    if seg == NSEG - 1:
                        # reload the accumulated partial once, before the last
                        # segment; start=True zeroes the bank and reopens the
                        # accumulation group. Earlier segments start fresh and
                        # are folded into usp by the spill-accumulate instead.
                        nc.tensor.matmul(
                            u, ident_r, usp[:, p],
                            start=True, stop=False, skip_group_check=True,
                        )
                if first_pv_of_seg[J] == idx:
                    drain_until(("v", seg, J % 4))
                et = et_tiles.pop(idx)
                u = u_tiles[p]
                for hx in range(2):
                    h = 2 * hp + hx
                    for it in range(4):
                        nc.tensor.matmul(
                            u[:, (hx * 4 + it) * DK : (hx * 4 + it + 1) * DK],
                            et[:, hx * IB + it * P : hx * IB + (it + 1) * P],
                            vt_sb[:, J, h * DK : (h + 1) * DK],
                            start=(first and hx == 0 and it == 0),
                            stop=(seg_end and hx == 1 and it == 3),
                            skip_group_check=True,
                        )
                        nc.tensor.matmul(
                            z_ps[:, p, hx * 4 + it : hx * 4 + it + 1],
                            et[:, hx * IB + it * P : hx * IB + (it + 1) * P],
                            ones_sb,
                            start=False,
                            stop=(last and hx == 1 and it == 3),
                            skip_group_check=True,
                        )
                if seg_end:
                    if last:
                        finish_pair(p, ib, hp)
                    else:
                        spill_pair(p)

            def spill_pair(p):
                u = u_tiles.pop(p)
                if p in spilled:
                    nc.vector.tensor_tensor(
                        usp[:, p], u.bitcast(F32R), usp[:, p], op=ADD
                    )
                else:
                    nc.vector.tensor_copy(usp[:, p], u)
                    spilled.add(p)

            def finish_pair(p, ib, hp):
                u = u_tiles.pop(p)
                rz = rzp.tile([P, 8, 1], F32, tag="rz", name="rz")
                nc.vector.reciprocal(rz, z_ps[:, p].rearrange("p (a o) -> p a o", o=1))
                # single strided multiply for the whole pair:
                # u is (hx, it, 64); o_sb wants (it, h, 64) -- express the
                # output with reordered dims so one op covers both heads
                o_pair = o_sb[:, ib * 4 : ib * 4 + 4, 2 * hp * DK : (2 * hp + 2) * DK]
                nc.vector.tensor_tensor(
                    o_pair.rearrange("p r (x c) -> p x r c", x=2),
                    u.rearrange("p (x r c) -> p x r c", x=2, r=4),
                    rz.rearrange("p (x r) o -> p x r o", x=2).broadcast_to(
                        [P, 2, 4, DK]
                    ),
                    op=MULT,
                )
                late = ib == NIB - 1
                if hp == 1 and late:
                    for pair in range(2):
                        work.append(
                            lambda pair=pair: transp2(ib, 1, pair, True)
                        )
                        for it in range(ib * 4 + 2 * pair, ib * 4 + 2 * pair + 2):
                            for nh in range(2):
                                work.append(
                                    lambda it=it, nh=nh: oproj(it, nh, True)
                                )
                else:
                    work.extend(
                        [
                            lambda pair=pair, late=late: transp2(ib, hp, pair, late)
                            for pair in range(2)
                        ]
                    )
                    if hp == 1:
                        for it in range(ib * 4, ib * 4 + 4):
                            for nh in range(2):
                                work.append(
                                    lambda it=it, nh=nh, late=late: oproj(it, nh, late)
                                )

            for idx in range(len(seq) + 2):
                if idx < len(seq):
                    emit_st_exp(idx)
                if idx >= 2:
                    emit_pv(idx - 2)
                drain(idx, force=(idx < 20 or idx >= len(seq) - 20))

            while work:
                work.popleft()()

    nc.compile()
    return nc


def _get_nc():
    global _NC_CACHE
    if _NC_CACHE is None:
        _NC_CACHE = _build()
    return _NC_CACHE


def _bf16(a):
    return np.ascontiguousarray(a).astype(ml_dtypes.bfloat16)


def _in_maps(query, key, value, wq, wk, wv, wo, bq, bk):
    maps = []
    for c in range(8):
        b, g = divmod(c, 4)
        sl = slice(g * DC, (g + 1) * DC)
        maps.append(
            {
                "xq_t": _bf16(query[:, b, :].T),
                "xk_t": _bf16(key[:, b, :].T),
                "xv_t": _bf16(value[:, b, :].T),
                "wq_t": _bf16(wq[sl, :].T),
                "wk_t": _bf16(wk[sl, :].T),
                "wv_t": _bf16(wv[sl, :].T),
                "wo_t": _bf16(wo[:, sl].T),
                "bqk_s": np.ascontiguousarray(
                    np.concatenate(
                        [bq[sl].reshape(2, P).T, bk[sl].reshape(2, P).T], axis=1
                    )
                ),
            }
        )
    return maps


def kernel(
    query, key, value, wq, bq, wk, bk, wv, bv, wo, bo, **_kw
) -> np.ndarray:
    query = np.asarray(query, np.float32)
    key = np.asarray(key, np.float32)
    value = np.asarray(value, np.float32)
    wq = np.asarray(wq, np.float32)
    wk = np.asarray(wk, np.float32)
    wv = np.asarray(wv, np.float32)
    wo = np.asarray(wo, np.float32)
    bq = np.asarray(bq, np.float32)
    bk = np.asarray(bk, np.float32)
    bv = np.asarray(bv, np.float32)
    bo = np.asarray(bo, np.float32)

    nc = _get_nc()
    res = run_bass_kernel_spmd(
        nc, _in_maps(query, key, value, wq, wk, wv, wo, bq, bk),
        core_ids=list(range(8)),
    )

    out = np.zeros((S, B, D), np.float32)
    for c in range(8):
        out[:, c // 4, :] += np.asarray(res.results[c]["y"]).astype(np.float32)
    out += bo + wo @ bv
    return out
